# revision 6
# baseline (speedup 1.0000x reference)
"""Depthwise causal Conv1d (k=4) + SiLU on 8 Trainium2 NeuronCores.

Problem: x [4, 4096, 2048] f32, w [2048, 4] f32,
out[b, t, d] = silu(sum_j w[d, j] * x[b, t - 3 + j, d])   (zero-padded left).

Sharding: 8 cores = 4 batches x 2 channel-halves. Depthwise conv is
independent per channel, so channel sharding needs no halo exchange.

Layout: each core receives its shard host-transposed to [channels, time]
(channels on SBUF partitions). The per-channel weight w[d, j] is a
per-partition scalar and the causal time shifts are free-dim AP offsets
into one loaded tile.

The kernel is HBM-bandwidth-bound (~16.8 MB of fp16 I/O per core against
~390 GB/s effective per-NeuronCore HBM), with compute only just fitting
under the DMA window, so the schedule works both angles:
 - All 8 input-block loads are queued back to back at the head of the
   sync HWDGE ring; stores are issued on the SAME ring afterwards. The
   ring drains FIFO, which gives loads strict priority over stores
   (loads finish at ~30 us) and keeps the SDMA engines fed to the end.
 - Compute is spread over FOUR engine lanes so no engine exceeds the
   DMA window: odd blocks run on the TensorEngine as diag(w_j) matmuls
   accumulating the 4 taps in PSUM (diag built on-chip from a 32 KB
   host-sent identity); even blocks run elementwise with the products
   split between DVE and GpSimd (GpSimd is otherwise idle since stores
   left the SWDGE path) and the pair-add tree on DVE. ACT does only
   SiLU.

Measured op rates (fp16): DVE tensor_scalar 0.29 ns/elem, DVE
tensor_tensor 0.54, GpSimd tensor_scalar ~0.9, ACT ~1.0, PE ~9.3 us per
4-tap 4096-col block.

Precision: x and the output are host-cast fp16 (halves HBM traffic both
ways); products and adds stay fp16 (PE accumulates fp32 in PSUM); SiLU
computes fp32-internally on ACT. End-to-end relative error ~5e-4.
"""

import sys
import types

import numpy as np

import concourse.bass as bass
import concourse.bacc as bacc
import concourse.mybir as mybir
from concourse.tile import TileContext
from concourse.bass_utils import run_bass_kernel_spmd


def _ensure_ntff_hook():
    """bass_utils imports antenv.axon_hooks when BASS_TRACE is set; that
    module is absent on this image. Install a shim so tracing works when
    possible and degrades gracefully (instead of crashing) when not."""
    try:
        import antenv.axon_hooks  # noqa: F401

        return
    except ImportError:
        pass
    try:
        import antenv

        hook = None
        try:
            if "/root/.axon_site" not in sys.path:
                sys.path.insert(0, "/root/.axon_site")
            from trn_agent_boot.trn_boot import _ntff_profile_via_ctypes

            hook = _ntff_profile_via_ctypes("/opt/axon/libaxon_pjrt.so")
        except Exception:
            hook = None
        mod = types.ModuleType("antenv.axon_hooks")
        mod._hook = hook
        mod.get_axon_ntff_profile_hook = lambda: mod._hook
        mod.set_axon_ntff_profile_hook = lambda h: setattr(mod, "_hook", h)
        sys.modules["antenv.axon_hooks"] = mod
        antenv.axon_hooks = mod
    except Exception:
        pass


_ensure_ntff_hook()

B, L, D = 4, 4096, 2048
K = 4
PAD = K - 1
N_CORES = 8
DH = D // 2            # channels per core
NBLK = DH // 128       # 128-partition channel blocks per core
ROWW = 4128            # DRAM row stride (fp16 elems): 64B-aligned rows

MID_DT = mybir.dt.float16
PE_BLKS = (1, 3, 5, 7)  # blocks computed on the TensorEngine

_cache = {}


def _build_bass():
    nc = bacc.Bacc()
    xt = nc.dram_tensor("xt", [DH, ROWW], MID_DT, kind="ExternalInput")
    wt = nc.dram_tensor("wt", [128, NBLK * K], mybir.dt.float32, kind="ExternalInput")
    ident = nc.dram_tensor("ident", [128, 128], MID_DT, kind="ExternalInput")
    ot = nc.dram_tensor("ot", [DH, L], MID_DT, kind="ExternalOutput")
    f32 = mybir.dt.float32
    HALF = L // 2

    with TileContext(nc) as tc:
        with tc.tile_pool(name="pool", bufs=2) as pool, \
             tc.tile_pool(name="psum", bufs=2, space="PSUM") as psum_pool:
            # w and the identity lead the sync ring so compute can start as
            # soon as the first x block lands.
            w = pool.tile([128, NBLK * K], f32, tag="w", bufs=1)
            nc.sync.dma_start(out=w[:], in_=wt[:, :])
            idt = pool.tile([128, 128], MID_DT, tag="idt", bufs=1)
            nc.sync.dma_start(out=idt[:], in_=ident[:, :])
            # Warmup: a tiny Silu forces the silu activation-table set to
            # load during the initial DMA wait; it is the only table load
            # in the whole kernel.
            warm = pool.tile([128, 2], MID_DT, tag="warm", bufs=1)
            nc.vector.memset(warm[:], 0.0)
            nc.scalar.activation(warm[:], warm[:], mybir.ActivationFunctionType.Silu)

            # All loads up front, back to back on the sync ring.
            xts = []
            for blk in range(NBLK):
                x = pool.tile([128, L + PAD + 1], MID_DT, tag="x", bufs=NBLK)
                nc.sync.dma_start(
                    out=x[:, 0 : L + PAD],
                    in_=xt[blk * 128 : (blk + 1) * 128, 0 : L + PAD],
                )
                xts.append(x)

            # Build diag(w_j) for the PE blocks on-chip: one per-partition
            # scaling of the identity each.
            wdt = pool.tile([128, len(PE_BLKS) * K * 128], MID_DT, tag="wd", bufs=1)
            wd_col = {}
            c = 0
            for blk in PE_BLKS:
                for j in range(K):
                    nc.vector.tensor_scalar_mul(
                        wdt[:, c : c + 128], idt[:], w[:, blk * K + j : blk * K + j + 1]
                    )
                    wd_col[(blk, j)] = c
                    c += 128

            outs = []
            for blk in range(NBLK):
                x = xts[blk]
                wj = lambda j: w[:, blk * K + j : blk * K + j + 1]
                o = pool.tile([128, L], MID_DT, tag="o", bufs=NBLK)
                if blk in PE_BLKS:
                    # TensorEngine path: per 1024-col PSUM quarter,
                    # accumulate the 4 diag(w_j) matmuls (shift = free-dim
                    # offset on the moving operand), SiLU out of PSUM.
                    PQ = 1024
                    for q in range(L // PQ):
                        h0 = q * PQ
                        ps = psum_pool.tile([128, PQ], f32, tag="ps", bufs=4)
                        for j in range(K):
                            lw = wdt[:, wd_col[(blk, j)] : wd_col[(blk, j)] + 128]
                            for cc in range(PQ // 512):
                                nc.tensor.matmul(
                                    ps[:, cc * 512 : (cc + 1) * 512],
                                    lw,
                                    x[:, h0 + cc * 512 + j : h0 + cc * 512 + j + 512],
                                    start=(j == 0),
                                    stop=(j == K - 1),
                                )
                        nc.scalar.activation(
                            o[:, h0 : h0 + PQ], ps[:],
                            mybir.ActivationFunctionType.Silu,
                        )
                else:
                    # Elementwise path: qe holds the even-shift products
                    # [q0 | q2] (DVE), qo the odd [q1 | q3] (GpSimd — the
                    # 4th otherwise-idle compute lane), each as one
                    # contiguous [128, 2, L] tile so both pair-adds run as
                    # a single DVE tensor_tensor op. Products are
                    # shift-rebased: q_j[:, t] = w_j * x[:, t + j].
                    qe = pool.tile([128, 2, L], MID_DT, tag="qe", bufs=2)
                    qo = pool.tile([128, 2, L], MID_DT, tag="qo", bufs=2)
                    nc.gpsimd.tensor_scalar_mul(qo[:, 0, :], x[:, 1 : 1 + L], wj(1))
                    nc.vector.tensor_scalar_mul(qe[:, 0, :], x[:, 0:L], wj(0))
                    nc.gpsimd.tensor_scalar_mul(qo[:, 1, :], x[:, 3 : 3 + L], wj(3))
                    nc.vector.tensor_scalar_mul(qe[:, 1, :], x[:, 2 : 2 + L], wj(2))
                    nc.vector.tensor_add(qe[:, :, :], qe[:, :, :], qo[:, :, :])
                    # final add + SiLU in halves so ACT can start early
                    for h in range(2):
                        s = h * HALF
                        nc.vector.tensor_add(
                            qe[:, 0, s : s + HALF],
                            qe[:, 0, s : s + HALF],
                            qe[:, 1, s : s + HALF],
                        )
                        nc.scalar.activation(
                            o[:, s : s + HALF], qe[:, 0, s : s + HALF],
                            mybir.ActivationFunctionType.Silu,
                        )
                outs.append(o)

            # Stores ride the tail of the sync ring: the ring drains FIFO,
            # so every load completes before the first store byte moves —
            # strict load priority with zero idle in between.
            for blk in range(NBLK):
                nc.sync.dma_start(
                    out=ot[blk * 128 : (blk + 1) * 128, 0:L], in_=outs[blk][:]
                )
    nc.compile()
    return nc


def _shard_inputs(x, w):
    ident = np.ascontiguousarray(np.eye(128, dtype=np.float16))
    in_maps = []
    for core in range(N_CORES):
        b, half = divmod(core, 2)
        d0 = half * DH
        xt = np.zeros((DH, ROWW), dtype=np.float16)
        xt[:, PAD : PAD + L] = x[b, :, d0 : d0 + DH].T.astype(np.float16)
        # w rows for this shard, rearranged so partition p holds the K
        # weights of channel blk*128 + p at free cols [blk*K, blk*K + K)
        w_sh = w[d0 : d0 + DH].reshape(NBLK, 128, K)
        wt = (
            w_sh.transpose(1, 0, 2).reshape(128, NBLK * K).astype(np.float32)
        )
        in_maps.append(
            {
                "xt": np.ascontiguousarray(xt),
                "wt": np.ascontiguousarray(wt),
                "ident": ident,
            }
        )
    return in_maps


def kernel(x, w):
    x = np.asarray(x, dtype=np.float32)
    w = np.asarray(w, dtype=np.float32)
    assert x.shape == (B, L, D) and w.shape == (D, K)

    if "nc" not in _cache:
        _cache["nc"] = _build_bass()
    nc = _cache["nc"]

    in_maps = _shard_inputs(x, w)
    res = None
    for attempt in range(3):
        try:
            res = run_bass_kernel_spmd(nc, in_maps, core_ids=list(range(N_CORES)))
            break
        except Exception:
            if attempt == 2:
                raise
    _cache["last_results"] = res

    out = np.empty((B, L, D), dtype=np.float32)
    for core in range(N_CORES):
        b, half = divmod(core, 2)
        d0 = half * DH
        out[b, :, d0 : d0 + DH] = res.results[core]["ot"].T.astype(np.float32)
    return out


# revision 8
# speedup vs baseline: 7.6554x; 7.6554x over previous
"""Depthwise causal Conv1d (k=4) + SiLU on 8 Trainium2 NeuronCores.

Problem: x [4, 4096, 2048] f32, w [2048, 4] f32,
out[b, t, d] = silu(sum_j w[d, j] * x[b, t - 3 + j, d])   (zero-padded left).

Sharding: 8 cores = 4 batches x 2 channel-halves. Depthwise conv is
independent per channel, so channel sharding needs no halo exchange.

Layout: each core receives its shard host-transposed to [channels, time]
(channels on SBUF partitions). The per-channel weight w[d, j] is a
per-partition scalar and the causal time shifts are free-dim AP offsets
into one loaded tile.

The kernel is HBM-bandwidth-bound (~16.8 MB of fp16 I/O per core against
~390 GB/s effective per-NeuronCore HBM), with compute only just fitting
under the DMA window, so the schedule works both angles:
 - All 8 input-block loads are queued back to back at the head of the
   sync HWDGE ring; stores are issued on the SAME ring afterwards. The
   ring drains FIFO, which gives loads strict priority over stores
   (loads finish at ~30 us) and keeps the SDMA engines fed to the end.
 - Compute is spread over FOUR engine lanes so no engine exceeds the
   DMA window: odd blocks run on the TensorEngine as diag(w_j) matmuls
   accumulating the 4 taps in PSUM (diag built on-chip from a 32 KB
   host-sent identity); even blocks run elementwise with the products
   split between DVE and GpSimd (GpSimd is otherwise idle since stores
   left the SWDGE path) and the pair-add tree on DVE. ACT does only
   SiLU.

Measured op rates (fp16): DVE tensor_scalar 0.29 ns/elem, DVE
tensor_tensor 0.54, GpSimd tensor_scalar ~0.9, ACT ~1.0, PE ~9.3 us per
4-tap 4096-col block.

Precision: x and the output are host-cast fp16 (halves HBM traffic both
ways); products and adds stay fp16 (PE accumulates fp32 in PSUM); SiLU
computes fp32-internally on ACT. End-to-end relative error ~5e-4.
"""

import sys
import types

import numpy as np

import concourse.bass as bass
import concourse.bacc as bacc
import concourse.mybir as mybir
from concourse.tile import TileContext
from concourse.bass_utils import run_bass_kernel_spmd


def _ensure_ntff_hook():
    """bass_utils imports antenv.axon_hooks when BASS_TRACE is set; that
    module is absent on this image. Install a shim so tracing works when
    possible and degrades gracefully (instead of crashing) when not."""
    try:
        import antenv.axon_hooks  # noqa: F401

        return
    except ImportError:
        pass
    try:
        import antenv

        hook = None
        try:
            if "/root/.axon_site" not in sys.path:
                sys.path.insert(0, "/root/.axon_site")
            from trn_agent_boot.trn_boot import _ntff_profile_via_ctypes

            hook = _ntff_profile_via_ctypes("/opt/axon/libaxon_pjrt.so")
        except Exception:
            hook = None
        mod = types.ModuleType("antenv.axon_hooks")
        mod._hook = hook
        mod.get_axon_ntff_profile_hook = lambda: mod._hook
        mod.set_axon_ntff_profile_hook = lambda h: setattr(mod, "_hook", h)
        sys.modules["antenv.axon_hooks"] = mod
        antenv.axon_hooks = mod
    except Exception:
        pass


_ensure_ntff_hook()

B, L, D = 4, 4096, 2048
K = 4
PAD = K - 1
N_CORES = 8
DH = D // 2            # channels per core
NBLK = DH // 128       # 128-partition channel blocks per core
ROWW = 4128            # DRAM row stride (fp16 elems): 64B-aligned rows

MID_DT = mybir.dt.float16
PE_BLKS = (1, 3, 5, 7)  # blocks computed fully on the TensorEngine
PE_HALF_BLK = 6         # block whose second half also runs on the TensorEngine
ACT_PROD_BLKS = (0,)    # elementwise blocks whose odd products run on ACT

_cache = {}


def _build_bass():
    nc = bacc.Bacc()
    xt = nc.dram_tensor("xt", [DH, ROWW], MID_DT, kind="ExternalInput")
    wt = nc.dram_tensor("wt", [128, NBLK * K], mybir.dt.float32, kind="ExternalInput")
    ident = nc.dram_tensor("ident", [128, 128], MID_DT, kind="ExternalInput")
    ot = nc.dram_tensor("ot", [DH, L], MID_DT, kind="ExternalOutput")
    f32 = mybir.dt.float32
    HALF = L // 2

    with TileContext(nc) as tc:
        with tc.tile_pool(name="pool", bufs=2) as pool, \
             tc.tile_pool(name="psum", bufs=2, space="PSUM") as psum_pool:
            # w and the identity lead the sync ring so compute can start as
            # soon as the first x block lands.
            w = pool.tile([128, NBLK * K], f32, tag="w", bufs=1)
            nc.sync.dma_start(out=w[:], in_=wt[:, :])
            idt = pool.tile([128, 128], MID_DT, tag="idt", bufs=1)
            nc.sync.dma_start(out=idt[:], in_=ident[:, :])
            # Warmup: a tiny Silu forces the silu activation-table set to
            # load during the initial DMA wait; it is the only table load
            # in the whole kernel.
            warm = pool.tile([128, 2], MID_DT, tag="warm", bufs=1)
            nc.vector.memset(warm[:], 0.0)
            nc.scalar.activation(warm[:], warm[:], mybir.ActivationFunctionType.Silu)

            # All loads up front, back to back on the sync ring.
            xts = []
            for blk in range(NBLK):
                x = pool.tile([128, L + PAD + 1], MID_DT, tag="x", bufs=NBLK)
                nc.sync.dma_start(
                    out=x[:, 0 : L + PAD],
                    in_=xt[blk * 128 : (blk + 1) * 128, 0 : L + PAD],
                )
                xts.append(x)

            # Build diag(w_j) for the PE blocks on-chip: one per-partition
            # scaling of the identity each.
            pe_diag_blks = list(PE_BLKS) + [PE_HALF_BLK]
            wdt = pool.tile([128, len(pe_diag_blks) * K * 128], MID_DT, tag="wd", bufs=1)
            wd_col = {}
            c = 0
            for blk in pe_diag_blks:
                for j in range(K):
                    nc.vector.tensor_scalar_mul(
                        wdt[:, c : c + 128], idt[:], w[:, blk * K + j : blk * K + j + 1]
                    )
                    wd_col[(blk, j)] = c
                    c += 128

            def pe_half(blk, x, o, h0):
                # TensorEngine path for [h0, h0+2048): accumulate the 4
                # diag(w_j) matmuls per 512-col PSUM chunk (shift =
                # free-dim offset on the moving operand), SiLU from PSUM.
                ps = psum_pool.tile([128, HALF], f32, tag="ps", bufs=2)
                for j in range(K):
                    lw = wdt[:, wd_col[(blk, j)] : wd_col[(blk, j)] + 128]
                    for cc in range(HALF // 512):
                        nc.tensor.matmul(
                            ps[:, cc * 512 : (cc + 1) * 512],
                            lw,
                            x[:, h0 + cc * 512 + j : h0 + cc * 512 + j + 512],
                            start=(j == 0),
                            stop=(j == K - 1),
                        )
                nc.scalar.activation(
                    o[:, h0 : h0 + HALF], ps[:], mybir.ActivationFunctionType.Silu
                )

            def dve_half(blk, x, o, h0, qe, qo, act_odd):
                # Elementwise path for [h0, h0+2048): qe holds the
                # even-shift products [q0 | q2], qo the odd [q1 | q3]
                # (on ACT for ACT_PROD_BLKS to relieve DVE), pair-add +
                # final add + SiLU. Shift-rebased: q_j[:, t] = w_j*x[t+j].
                wj = lambda j: w[:, blk * K + j : blk * K + j + 1]
                if act_odd:
                    nc.scalar.mul(qo[:, 0, :], x[:, h0 + 1 : h0 + 1 + HALF], wj(1))
                else:
                    nc.vector.tensor_scalar_mul(
                        qo[:, 0, :], x[:, h0 + 1 : h0 + 1 + HALF], wj(1)
                    )
                nc.vector.tensor_scalar_mul(qe[:, 0, :], x[:, h0 : h0 + HALF], wj(0))
                if act_odd:
                    nc.scalar.mul(qo[:, 1, :], x[:, h0 + 3 : h0 + 3 + HALF], wj(3))
                else:
                    nc.vector.tensor_scalar_mul(
                        qo[:, 1, :], x[:, h0 + 3 : h0 + 3 + HALF], wj(3)
                    )
                nc.vector.tensor_scalar_mul(
                    qe[:, 1, :], x[:, h0 + 2 : h0 + 2 + HALF], wj(2)
                )
                nc.vector.tensor_add(qe[:, :, :], qe[:, :, :], qo[:, :, :])
                nc.vector.tensor_add(qe[:, 0, :], qe[:, 0, :], qe[:, 1, :])
                nc.scalar.activation(
                    o[:, h0 : h0 + HALF], qe[:, 0, :],
                    mybir.ActivationFunctionType.Silu,
                )

            outs = []
            for blk in range(NBLK):
                x = xts[blk]
                o = pool.tile([128, L], MID_DT, tag="o", bufs=NBLK)
                for h in range(2):
                    h0 = h * HALF
                    if blk in PE_BLKS or (blk == PE_HALF_BLK and h == 1):
                        pe_half(blk, x, o, h0)
                    else:
                        qe = pool.tile([128, 2, HALF], MID_DT, tag="qe", bufs=3)
                        qo = pool.tile([128, 2, HALF], MID_DT, tag="qo", bufs=3)
                        dve_half(blk, x, o, h0, qe, qo, blk in ACT_PROD_BLKS)
                    # Stores ride the tail of the sync ring: the ring
                    # drains FIFO, so loads keep strict priority and the
                    # ring never idles while work exists.
                    nc.sync.dma_start(
                        out=ot[blk * 128 : (blk + 1) * 128, h0 : h0 + HALF],
                        in_=o[:, h0 : h0 + HALF],
                    )
                outs.append(o)
    nc.compile()
    return nc


def _shard_inputs(x, w):
    ident = np.ascontiguousarray(np.eye(128, dtype=np.float16))
    in_maps = []
    for core in range(N_CORES):
        b, half = divmod(core, 2)
        d0 = half * DH
        xt = np.zeros((DH, ROWW), dtype=np.float16)
        xt[:, PAD : PAD + L] = x[b, :, d0 : d0 + DH].T.astype(np.float16)
        # w rows for this shard, rearranged so partition p holds the K
        # weights of channel blk*128 + p at free cols [blk*K, blk*K + K)
        w_sh = w[d0 : d0 + DH].reshape(NBLK, 128, K)
        wt = (
            w_sh.transpose(1, 0, 2).reshape(128, NBLK * K).astype(np.float32)
        )
        in_maps.append(
            {
                "xt": np.ascontiguousarray(xt),
                "wt": np.ascontiguousarray(wt),
                "ident": ident,
            }
        )
    return in_maps


def kernel(x, w):
    x = np.asarray(x, dtype=np.float32)
    w = np.asarray(w, dtype=np.float32)
    assert x.shape == (B, L, D) and w.shape == (D, K)

    if "nc" not in _cache:
        _cache["nc"] = _build_bass()
    nc = _cache["nc"]

    in_maps = _shard_inputs(x, w)
    res = None
    for attempt in range(3):
        try:
            res = run_bass_kernel_spmd(nc, in_maps, core_ids=list(range(N_CORES)))
            break
        except Exception:
            if attempt == 2:
                raise
    _cache["last_results"] = res

    out = np.empty((B, L, D), dtype=np.float32)
    for core in range(N_CORES):
        b, half = divmod(core, 2)
        d0 = half * DH
        out[b, :, d0 : d0 + DH] = res.results[core]["ot"].T.astype(np.float32)
    return out


# revision 9
# speedup vs baseline: 7.8876x; 1.0303x over previous
"""Depthwise causal Conv1d (k=4) + SiLU on 8 Trainium2 NeuronCores.

Problem: x [4, 4096, 2048] f32, w [2048, 4] f32,
out[b, t, d] = silu(sum_j w[d, j] * x[b, t - 3 + j, d])   (zero-padded left).

Sharding: 8 cores = 4 batches x 2 channel-halves. Depthwise conv is
independent per channel, so channel sharding needs no halo exchange.

Layout: each core receives its shard host-transposed to [channels, time]
(channels on SBUF partitions). The per-channel weight w[d, j] is a
per-partition scalar and the causal time shifts are free-dim AP offsets
into one loaded tile.

The kernel is HBM-bandwidth-bound (~16.8 MB of fp16 I/O per core against
~390 GB/s effective per-NeuronCore HBM), with compute only just fitting
under the DMA window, so the schedule works both angles:
 - All 8 input-block loads are queued back to back at the head of the
   sync HWDGE ring; stores are issued on the SAME ring afterwards. The
   ring drains FIFO, which gives loads strict priority over stores
   (loads finish at ~30 us) and keeps the SDMA engines fed to the end.
 - Compute is spread over FOUR engine lanes so no engine exceeds the
   DMA window: odd blocks run on the TensorEngine as diag(w_j) matmuls
   accumulating the 4 taps in PSUM (diag built on-chip from a 32 KB
   host-sent identity); even blocks run elementwise with the products
   split between DVE and GpSimd (GpSimd is otherwise idle since stores
   left the SWDGE path) and the pair-add tree on DVE. ACT does only
   SiLU.

Measured op rates (fp16): DVE tensor_scalar 0.29 ns/elem, DVE
tensor_tensor 0.54, GpSimd tensor_scalar ~0.9, ACT ~1.0, PE ~9.3 us per
4-tap 4096-col block.

Precision: x and the output are host-cast fp16 (halves HBM traffic both
ways); products and adds stay fp16 (PE accumulates fp32 in PSUM); SiLU
computes fp32-internally on ACT. End-to-end relative error ~5e-4.
"""

import sys
import types

import numpy as np

import concourse.bass as bass
import concourse.bacc as bacc
import concourse.mybir as mybir
from concourse.tile import TileContext
from concourse.bass_utils import run_bass_kernel_spmd


def _ensure_ntff_hook():
    """bass_utils imports antenv.axon_hooks when BASS_TRACE is set; that
    module is absent on this image. Install a shim so tracing works when
    possible and degrades gracefully (instead of crashing) when not."""
    try:
        import antenv.axon_hooks  # noqa: F401

        return
    except ImportError:
        pass
    try:
        import antenv

        hook = None
        try:
            if "/root/.axon_site" not in sys.path:
                sys.path.insert(0, "/root/.axon_site")
            from trn_agent_boot.trn_boot import _ntff_profile_via_ctypes

            hook = _ntff_profile_via_ctypes("/opt/axon/libaxon_pjrt.so")
        except Exception:
            hook = None
        mod = types.ModuleType("antenv.axon_hooks")
        mod._hook = hook
        mod.get_axon_ntff_profile_hook = lambda: mod._hook
        mod.set_axon_ntff_profile_hook = lambda h: setattr(mod, "_hook", h)
        sys.modules["antenv.axon_hooks"] = mod
        antenv.axon_hooks = mod
    except Exception:
        pass


_ensure_ntff_hook()

B, L, D = 4, 4096, 2048
K = 4
PAD = K - 1
N_CORES = 8
DH = D // 2            # channels per core
NBLK = DH // 128       # 128-partition channel blocks per core
ROWW = 4128            # DRAM row stride (fp16 elems): 64B-aligned rows

MID_DT = mybir.dt.float16
PE_BLKS = (1, 3, 5, 7)  # blocks computed fully on the TensorEngine
PE_HALF_BLK = 6         # block whose second half also runs on the TensorEngine
ACT_PROD_BLKS = (0,)    # elementwise blocks whose odd products run on ACT

_cache = {}


def _build_bass():
    nc = bacc.Bacc()
    xt = nc.dram_tensor("xt", [DH, ROWW], MID_DT, kind="ExternalInput")
    wt = nc.dram_tensor("wt", [128, NBLK * K], mybir.dt.float32, kind="ExternalInput")
    ident = nc.dram_tensor("ident", [128, 128], MID_DT, kind="ExternalInput")
    ot = nc.dram_tensor("ot", [DH, L], MID_DT, kind="ExternalOutput")
    f32 = mybir.dt.float32
    HALF = L // 2

    with TileContext(nc) as tc:
        with tc.tile_pool(name="pool", bufs=2) as pool, \
             tc.tile_pool(name="psum", bufs=2, space="PSUM") as psum_pool:
            # w and the identity lead the sync ring so compute can start as
            # soon as the first x block lands.
            w = pool.tile([128, NBLK * K], f32, tag="w", bufs=1)
            nc.sync.dma_start(out=w[:], in_=wt[:, :])
            idt = pool.tile([128, 128], MID_DT, tag="idt", bufs=1)
            nc.sync.dma_start(out=idt[:], in_=ident[:, :])
            # Warmup: a tiny Silu forces the silu activation-table set to
            # load during the initial DMA wait; it is the only table load
            # in the whole kernel.
            warm = pool.tile([128, 2], MID_DT, tag="warm", bufs=1)
            nc.vector.memset(warm[:], 0.0)
            nc.scalar.activation(warm[:], warm[:], mybir.ActivationFunctionType.Silu)

            # All loads up front, back to back on the sync ring.
            xts = []
            for blk in range(NBLK):
                x = pool.tile([128, L + PAD + 1], MID_DT, tag="x", bufs=NBLK)
                nc.sync.dma_start(
                    out=x[:, 0 : L + PAD],
                    in_=xt[blk * 128 : (blk + 1) * 128, 0 : L + PAD],
                )
                xts.append(x)

            # Build diag(w_j) for the PE blocks on-chip: one per-partition
            # scaling of the identity each.
            pe_diag_blks = list(PE_BLKS) + [PE_HALF_BLK]
            wdt = pool.tile([128, len(pe_diag_blks) * K * 128], MID_DT, tag="wd", bufs=1)
            wd_col = {}
            c = 0
            for blk in pe_diag_blks:
                for j in range(K):
                    nc.vector.tensor_scalar_mul(
                        wdt[:, c : c + 128], idt[:], w[:, blk * K + j : blk * K + j + 1]
                    )
                    wd_col[(blk, j)] = c
                    c += 128

            def pe_half(blk, x, o, h0):
                # TensorEngine path for [h0, h0+2048): accumulate the 4
                # diag(w_j) matmuls per 512-col PSUM chunk (shift =
                # free-dim offset on the moving operand), SiLU from PSUM.
                # 1024-col PSUM quarters, 4 deep, so the PE stays 3 ahead
                # of ACT's PSUM drain.
                PQ = 1024
                for q in range(2):
                    q0 = h0 + q * PQ
                    ps = psum_pool.tile([128, PQ], f32, tag="ps", bufs=4)
                    for j in range(K):
                        lw = wdt[:, wd_col[(blk, j)] : wd_col[(blk, j)] + 128]
                        for cc in range(PQ // 512):
                            nc.tensor.matmul(
                                ps[:, cc * 512 : (cc + 1) * 512],
                                lw,
                                x[:, q0 + cc * 512 + j : q0 + cc * 512 + j + 512],
                                start=(j == 0),
                                stop=(j == K - 1),
                            )
                    nc.scalar.activation(
                        o[:, q0 : q0 + PQ], ps[:], mybir.ActivationFunctionType.Silu
                    )

            def dve_half(blk, x, o, h0, qe, qo, act_odd):
                # Elementwise path for [h0, h0+2048): qe holds the
                # even-shift products [q0 | q2], qo the odd [q1 | q3]
                # (on ACT for ACT_PROD_BLKS to relieve DVE), pair-add +
                # final add + SiLU. Shift-rebased: q_j[:, t] = w_j*x[t+j].
                wj = lambda j: w[:, blk * K + j : blk * K + j + 1]
                if act_odd:
                    nc.scalar.mul(qo[:, 0, :], x[:, h0 + 1 : h0 + 1 + HALF], wj(1))
                else:
                    nc.vector.tensor_scalar_mul(
                        qo[:, 0, :], x[:, h0 + 1 : h0 + 1 + HALF], wj(1)
                    )
                nc.vector.tensor_scalar_mul(qe[:, 0, :], x[:, h0 : h0 + HALF], wj(0))
                if act_odd:
                    nc.scalar.mul(qo[:, 1, :], x[:, h0 + 3 : h0 + 3 + HALF], wj(3))
                else:
                    nc.vector.tensor_scalar_mul(
                        qo[:, 1, :], x[:, h0 + 3 : h0 + 3 + HALF], wj(3)
                    )
                nc.vector.tensor_scalar_mul(
                    qe[:, 1, :], x[:, h0 + 2 : h0 + 2 + HALF], wj(2)
                )
                nc.vector.tensor_add(qe[:, :, :], qe[:, :, :], qo[:, :, :])
                nc.vector.tensor_add(qe[:, 0, :], qe[:, 0, :], qe[:, 1, :])
                nc.scalar.activation(
                    o[:, h0 : h0 + HALF], qe[:, 0, :],
                    mybir.ActivationFunctionType.Silu,
                )

            outs = []
            for blk in range(NBLK):
                x = xts[blk]
                o = pool.tile([128, L], MID_DT, tag="o", bufs=NBLK)
                for h in range(2):
                    h0 = h * HALF
                    if blk in PE_BLKS or (blk == PE_HALF_BLK and h == 1):
                        pe_half(blk, x, o, h0)
                    else:
                        qe = pool.tile([128, 2, HALF], MID_DT, tag="qe", bufs=3)
                        qo = pool.tile([128, 2, HALF], MID_DT, tag="qo", bufs=3)
                        dve_half(blk, x, o, h0, qe, qo, blk in ACT_PROD_BLKS)
                    # Stores ride the tail of the sync ring: the ring
                    # drains FIFO, so loads keep strict priority and the
                    # ring never idles while work exists.
                    nc.sync.dma_start(
                        out=ot[blk * 128 : (blk + 1) * 128, h0 : h0 + HALF],
                        in_=o[:, h0 : h0 + HALF],
                    )
                outs.append(o)
    nc.compile()
    return nc


def _shard_inputs(x, w):
    ident = np.ascontiguousarray(np.eye(128, dtype=np.float16))
    in_maps = []
    for core in range(N_CORES):
        b, half = divmod(core, 2)
        d0 = half * DH
        xt = np.zeros((DH, ROWW), dtype=np.float16)
        xt[:, PAD : PAD + L] = x[b, :, d0 : d0 + DH].T.astype(np.float16)
        # w rows for this shard, rearranged so partition p holds the K
        # weights of channel blk*128 + p at free cols [blk*K, blk*K + K)
        w_sh = w[d0 : d0 + DH].reshape(NBLK, 128, K)
        wt = (
            w_sh.transpose(1, 0, 2).reshape(128, NBLK * K).astype(np.float32)
        )
        in_maps.append(
            {
                "xt": np.ascontiguousarray(xt),
                "wt": np.ascontiguousarray(wt),
                "ident": ident,
            }
        )
    return in_maps


def kernel(x, w):
    x = np.asarray(x, dtype=np.float32)
    w = np.asarray(w, dtype=np.float32)
    assert x.shape == (B, L, D) and w.shape == (D, K)

    if "nc" not in _cache:
        _cache["nc"] = _build_bass()
    nc = _cache["nc"]

    in_maps = _shard_inputs(x, w)
    res = None
    for attempt in range(3):
        try:
            res = run_bass_kernel_spmd(nc, in_maps, core_ids=list(range(N_CORES)))
            break
        except Exception:
            if attempt == 2:
                raise
    _cache["last_results"] = res

    out = np.empty((B, L, D), dtype=np.float32)
    for core in range(N_CORES):
        b, half = divmod(core, 2)
        d0 = half * DH
        out[b, :, d0 : d0 + DH] = res.results[core]["ot"].T.astype(np.float32)
    return out
